# revision 1
# baseline (speedup 1.0000x reference)
"""BGAT attention kernel for Trainium2 (8 NeuronCores, batch-parallel).

Strategy (per core = one batch element):
  score[u,a,k] = (1/8) * sum_d av[k,d] * lrelu(S), S = (U+A+E)[u,a,(k,d)]
  Using lrelu(x) = 0.6x + 0.4|x|:
    score = T1 + sum_pos |S''| - sum_neg |S''|
  where S'' has per-column weights folded with 0.4/8*|av_d| (columns permuted
  so each head's positive-av columns sit in one padded uniform block, negative
  in another), and T1 = linear term via folded projection columns (exact).
  E-term weights ride a K=65 augmented matmul (ones row x U[u] row) so the
  per-user broadcast add is free; the A-term rides an identity matmul into the
  same PSUM accumulation.
  softmax needs no max-subtraction (scores are tiny by construction).
  Message sums commute with the edge projection:
    sum_a alpha*E = (sum_a alpha*edge) @ We   (and same over u)
  so phase 3 is small matmuls over natural-layout edge tiles.
"""

import math
from contextlib import ExitStack

import numpy as np

# ---- problem sizes (hardcoded from spec) ----
B = 8
FULL_CFG = dict(NU=256, NA=256, ED=64, UD=128, AD=128, H=8, HD=64)
SLOPE = 0.2


def make_cfg(NU, NA, ED, UD, AD, H, HD, av, UC=None):
    """Host-side layout metadata derived from av sign pattern."""
    cfg = dict(NU=NU, NA=NA, ED=ED, UD=UD, AD=AD, H=H, HD=HD)
    cfg["HH"] = H * HD
    scale = 1.0 / math.sqrt(HD)
    av = np.asarray(av, np.float32)
    pos_idx = [np.nonzero(av[k] >= 0)[0] for k in range(H)]
    neg_idx = [np.nonzero(av[k] < 0)[0] for k in range(H)]
    P_ = max(len(ix) for ix in pos_idx)
    N_ = max(len(ix) for ix in neg_idx)
    cfg["P_"], cfg["N_"] = P_, N_
    cfg["EXTC"] = H * P_ + H * N_ + H
    cfg["pos_idx"], cfg["neg_idx"] = pos_idx, neg_idx
    cfg["scale"] = scale
    cfg["NAH"] = (NA + 127) // 128  # number of 128-wide antenna chunks
    cfg["ACH"] = min(128, NA)
    cfg["UC"] = min(128, NU) if UC is None else UC
    cfg["NUC"] = NU // cfg["UC"]  # number of user chunks
    assert NU % 8 == 0
    cfg["NG"] = NU // 8  # softmax groups of 8 users
    return cfg


def prep_weights(Wu, Wa, We, av, Wres, cfg):
    """Build folded/permuted weight blocks. Returns dict of np arrays."""
    H, HD, ED, UD, AD = cfg["H"], cfg["HD"], cfg["ED"], cfg["UD"], cfg["AD"]
    P_, N_, EXTC, HH = cfg["P_"], cfg["N_"], cfg["EXTC"], cfg["HH"]
    scale = cfg["scale"]
    Wu, Wa, We = (np.asarray(x, np.float32) for x in (Wu, Wa, We))
    av = np.asarray(av, np.float32)
    Wres = np.asarray(Wres, np.float32)

    wu_big = np.zeros((UD, EXTC + HH), np.float32)
    wa_big = np.zeros((AD, EXTC + HH), np.float32)
    we_big = np.zeros((ED, EXTC + HH), np.float32)
    for k in range(H):
        for i, d in enumerate(cfg["pos_idx"][k]):
            c = 0.4 * scale * abs(av[k, d])
            col = k * P_ + i
            wu_big[:, col] = Wu[k][:, d] * c
            wa_big[:, col] = Wa[k][:, d] * c
            we_big[:, col] = We[k][:, d] * c
        for i, d in enumerate(cfg["neg_idx"][k]):
            c = 0.4 * scale * abs(av[k, d])
            col = H * P_ + k * N_ + i
            wu_big[:, col] = Wu[k][:, d] * c
            wa_big[:, col] = Wa[k][:, d] * c
            we_big[:, col] = We[k][:, d] * c
        # T1 (linear) columns: W @ (0.6*scale*av_k)
        t1w = 0.6 * scale * av[k]
        col = H * P_ + H * N_ + k
        wu_big[:, col] = Wu[k] @ t1w
        wa_big[:, col] = Wa[k] @ t1w
        we_big[:, col] = We[k] @ t1w
        # raw blocks for message matmuls
        wu_big[:, EXTC + k * HD : EXTC + (k + 1) * HD] = Wu[k]
        wa_big[:, EXTC + k * HD : EXTC + (k + 1) * HD] = Wa[k]
        we_big[:, EXTC + k * HD : EXTC + (k + 1) * HD] = We[k]

    ident = np.eye(128, dtype=np.float32)
    return dict(wu_big=wu_big, wa_big=wa_big, we_big=we_big, wres=Wres,
                ident=ident)


def build_bgat(ctx: ExitStack, tc, outs, ins, cfg):
    """Emit the Tile program. outs/ins: dicts name->AP."""
    import concourse.bass as bass
    import concourse.mybir as mybir

    nc = tc.nc
    f32 = mybir.dt.float32
    AX = mybir.AxisListType.X
    ADD = mybir.AluOpType.add
    EXPF = mybir.ActivationFunctionType.Exp

    NU, NA, ED, UD, AD = cfg["NU"], cfg["NA"], cfg["ED"], cfg["UD"], cfg["AD"]
    H, HD, HH = cfg["H"], cfg["HD"], cfg["HH"]
    P_, N_, EXTC = cfg["P_"], cfg["N_"], cfg["EXTC"]
    NAH, ACH, UC, NUC, NG = cfg["NAH"], cfg["ACH"], cfg["UC"], cfg["NUC"], cfg["NG"]
    HIDDEN = HH
    POSW, NEGW = H * P_, H * N_

    edge = ins["edge"]      # [NU*NA, ED]
    user = ins["user"]      # [NU, UD]
    ant = ins["ant"]        # [NA, AD]
    wu_big_d = ins["wu_big"]
    wa_big_d = ins["wa_big"]
    we_big_d = ins["we_big"]
    wres_d = ins["wres"]
    ident_d = ins["ident"]
    user_out = outs["user_out"]  # [NU, HIDDEN]
    ant_out = outs["ant_out"]    # [NA, HIDDEN]

    # x-major chunked view of edge: chunk c has 128 consecutive (u,a) rows
    CH = ACH  # rows per chunk (128 at full size)
    n_chunks_per_u = NAH
    edge_x = edge.rearrange("(c p) e -> c p e", p=CH)
    # u-major view for ant-side: partition = user
    edge_u = edge.rearrange("(j p a) e -> j p (a e)", p=UC, a=NA)

    consts = ctx.enter_context(tc.tile_pool(name="consts", bufs=1))

    # ---------- persistent SBUF tensors ----------
    ident_sb = consts.tile([128, 128], f32)
    nc.sync.dma_start(ident_sb[:], ident_d[:, :])
    wu_big_sb = consts.tile([UD, EXTC + HH], f32)
    nc.sync.dma_start(wu_big_sb[:], wu_big_d[:, :])
    wa_big_sb = consts.tile([AD, EXTC + HH], f32)
    nc.sync.dma_start(wa_big_sb[:], wa_big_d[:, :])
    we_big_sb = consts.tile([ED, EXTC + HH], f32)
    nc.sync.dma_start(we_big_sb[:], we_big_d[:, :])
    wres_sb = consts.tile([UD, HIDDEN], f32)
    nc.sync.dma_start(wres_sb[:], wres_d[:, :])

    ones_col = consts.tile([128, 1], f32)
    nc.gpsimd.memset(ones_col[:], 1.0)
    ones_row = consts.tile([1, 128], f32)
    nc.gpsimd.memset(ones_row[:], 1.0)

    U_big = consts.tile([UC, NUC, EXTC + HH], f32)
    A_big = consts.tile([ACH, NAH, EXTC + HH], f32)
    userT = consts.tile([UD, NU], f32)
    antT = consts.tile([AD, NA], f32)
    # alpha layouts: v3 = antenna-major, head-outer; v2 = user-major
    alpha_v3 = consts.tile([ACH, NAH, H, NU], f32)
    alpha_v2 = consts.tile([UC, NUC, H, NA], f32)
    ew_all = consts.tile([ED, NU, H], f32)
    ewa_all = consts.tile([ED, NA, H], f32)

    # combo rhs tiles (rows 0..ED-1 = we_big ext cols, row ED = per-user U row)
    combo0 = consts.tile([ED + 1, EXTC], f32)
    combo1 = consts.tile([ED + 1, EXTC], f32)
    combos = [combo0, combo1]
    for cb in combos:
        nc.gpsimd.dma_start(cb[0:ED, :], we_big_d[:, 0:EXTC])

    # ---------- precompute: transposes and U/A projections ----------
    with tc.tile_pool(name="pre_sb", bufs=2) as pre_sb, \
         tc.tile_pool(name="pre_ps", bufs=2, space="PSUM") as pre_ps:
        # user/ant feature tiles and transposes
        for (feat, T_sb, n, fd) in ((user, userT, NU, UD), (ant, antT, NA, AD)):
            fv = feat.rearrange("(j p) f -> j p f", p=min(128, n))
            for j in range(fv.shape[0]):
                p = fv.shape[1]
                ft = pre_sb.tile([p, fd], f32, tag="ft")
                nc.sync.dma_start(ft[:], fv[j])
                pt = pre_ps.tile([fd, p], f32, tag="pt")
                nc.tensor.transpose(pt[:], ft[:], ident_sb[0:p, 0:p])
                nc.scalar.copy(T_sb[:, j * p : j * p + p], pt[:])
        # U_big / A_big
        for (T_sb, big, nchunk, pc, fd) in (
            (userT, U_big, NUC, UC, UD),
            (antT, A_big, NAH, ACH, AD),
        ):
            w_sb = wu_big_sb if big is U_big else wa_big_sb
            for j in range(nchunk):
                for c0 in range(0, EXTC + HH, 512):
                    c1 = min(c0 + 512, EXTC + HH)
                    ps = pre_ps.tile([pc, 512], f32, tag="proj")
                    nc.tensor.matmul(ps[:, 0 : c1 - c0],
                                     T_sb[:, j * pc : j * pc + pc],
                                     w_sb[:, c0:c1], start=True, stop=True)
                    nc.scalar.copy(big[:, j, c0:c1], ps[:, 0 : c1 - c0])

    # ---------- pass 1: scores + softmax + user-side weighted edge sums ----
    # psum_misc bank layout (per group of 8 users):
    T1_OFF = 0                      # [128, NAH*8*H]
    SUM_OFF = T1_OFF + NAH * 8 * H  # [1, 8*H]
    RB_OFF = SUM_OFF + 8 * H        # [128, 8*H]
    EW_OFF = RB_OFF + 8 * H         # [ED, 8*H]
    assert EW_OFF + 8 * H <= 512

    with tc.tile_pool(name="edge_pool", bufs=6 * NAH + 2 * 8 * NAH) as edge_pool, \
         tc.tile_pool(name="p1_sb", bufs=3) as p1_sb, \
         tc.tile_pool(name="p1_stage", bufs=2) as p1_stage, \
         tc.tile_pool(name="ps_pos", bufs=2, space="PSUM") as ps_pos_pool, \
         tc.tile_pool(name="ps_neg", bufs=2, space="PSUM") as ps_neg_pool, \
         tc.tile_pool(name="ps_tp", bufs=2, space="PSUM") as ps_tp_pool, \
         tc.tile_pool(name="ps_misc", bufs=2, space="PSUM") as ps_misc_pool:

        # chunk list per group, in emission order, processed in sub-batches
        # of 4 (4 transposes share one psum bank + one batched copy)
        assert (8 * NAH) % 4 == 0
        for g in range(NG):
            misc = ps_misc_pool.tile([128, 512], f32, tag="misc")
            stage_P = p1_stage.tile([ACH, NAH * 8, H], f32, tag="sP")
            stage_N = p1_stage.tile([ACH, NAH * 8, H], f32, tag="sN")
            edge_tiles = {}
            chunks = [(ui, h) for ui in range(8) for h in range(NAH)]
            for u4 in range(0, len(chunks), 4):
                batch = chunks[u4 : u4 + 4]
                tp = ps_tp_pool.tile([ED, 512], f32, tag="tp")
                edT4 = p1_sb.tile([ED + 1, 512], f32, tag="edT4")
                nc.vector.memset(edT4[ED : ED + 1, :], 1.0)
                for q, (ui, h) in enumerate(batch):
                    u = g * 8 + ui
                    c = u * n_chunks_per_u + h
                    et = edge_pool.tile([CH, ED], f32, tag="edge")
                    nc.sync.dma_start(et[:], edge_x[c])
                    edge_tiles[(ui, h)] = et
                    nc.tensor.transpose(tp[:, q * 128 : q * 128 + CH], et[:],
                                        ident_sb[0:CH, 0:CH])
                for q, (ui, h) in enumerate(batch):
                    if h == 0:
                        u = g * 8 + ui
                        cb = combos[u % 2]
                        # per-user U row into combo row ED
                        nc.gpsimd.dma_start(
                            cb[ED : ED + 1, :],
                            U_big[u % UC : u % UC + 1, u // UC, 0:EXTC])
                nc.scalar.copy(edT4[0:ED, :], tp[:, :])
                for q, (ui, h) in enumerate(batch):
                    u = g * 8 + ui
                    cb = combos[u % 2]
                    sidx = h * 8 + ui
                    t1s = misc[0:CH, T1_OFF + sidx * H : T1_OFF + (sidx + 1) * H]
                    lhs = edT4[0 : ED + 1, q * 128 : q * 128 + CH]
                    ps_pos = ps_pos_pool.tile([CH, 512], f32, tag="pos")
                    ps_neg = ps_neg_pool.tile([CH, 512], f32, tag="neg")
                    # E+U into psum (K=ED+1 augmented), then A via identity mm
                    nc.tensor.matmul(ps_pos[:, 0:POSW], lhs, cb[:, 0:POSW],
                                     start=True, stop=False)
                    nc.tensor.matmul(ps_neg[:, 0:NEGW], lhs,
                                     cb[:, POSW : POSW + NEGW],
                                     start=True, stop=False)
                    nc.tensor.matmul(t1s, lhs, cb[:, POSW + NEGW : EXTC],
                                     start=True, stop=False)
                    nc.tensor.matmul(ps_pos[:, 0:POSW], ident_sb[0:ACH, 0:ACH],
                                     A_big[:, h, 0:POSW], start=False, stop=True)
                    nc.tensor.matmul(ps_neg[:, 0:NEGW], ident_sb[0:ACH, 0:ACH],
                                     A_big[:, h, POSW : POSW + NEGW],
                                     start=False, stop=True)
                    nc.tensor.matmul(t1s, ident_sb[0:ACH, 0:ACH],
                                     A_big[:, h, POSW + NEGW : EXTC],
                                     start=False, stop=True)
                    # |.| reduces
                    nc.vector.tensor_reduce(
                        stage_P[:, sidx, :],
                        ps_pos[:, 0:POSW].rearrange("p (k d) -> p k d", d=P_),
                        axis=AX, op=ADD, apply_absolute_value=True)
                    nc.vector.tensor_reduce(
                        stage_N[:, sidx, :],
                        ps_neg[:, 0:NEGW].rearrange("p (k d) -> p k d", d=N_),
                        axis=AX, op=ADD, apply_absolute_value=True)

            # ---- group softmax ----
            # score_g memory order (h, u, k); exp_g memory order (h, k, u)
            gsz = NAH * 8 * H
            score_g = p1_sb.tile([ACH, gsz], f32, tag="score", bufs=4)
            nc.vector.tensor_sub(score_g[:],
                                 stage_P[:].rearrange("p a b -> p (a b)"),
                                 stage_N[:].rearrange("p a b -> p (a b)"))
            nc.vector.tensor_add(score_g[:], score_g[:],
                                 misc[0:ACH, T1_OFF : T1_OFF + gsz])
            exp_g = p1_sb.tile([ACH, gsz], f32, tag="expg", bufs=6)
            nc.scalar.activation(
                exp_g[:].rearrange("p (a c b) -> p a b c", a=NAH, c=H),
                score_g[:].rearrange("p (a b c) -> p a b c", a=NAH, b=8),
                EXPF)
            for h in range(NAH):
                nc.tensor.matmul(
                    misc[0:1, SUM_OFF : SUM_OFF + 8 * H], ones_col[0:ACH, :],
                    exp_g[:, h * 8 * H : (h + 1) * 8 * H],
                    start=(h == 0), stop=(h == NAH - 1))
            rec = p1_sb.tile([1, 8 * H], f32, tag="rec", bufs=4)
            nc.vector.reciprocal(rec[:], misc[0:1, SUM_OFF : SUM_OFF + 8 * H])
            nc.tensor.matmul(misc[0:128, RB_OFF : RB_OFF + 8 * H],
                             ones_row[:, 0:128], rec[:], start=True, stop=True)
            # alpha (normalized), kept in flat group tile + scattered to v3
            for h in range(NAH):
                sl = exp_g[:, h * 8 * H : (h + 1) * 8 * H]
                nc.vector.tensor_mul(sl, sl,
                                     misc[0:ACH, RB_OFF : RB_OFF + 8 * H])
                nc.vector.tensor_copy(
                    alpha_v3[:, h, :, g * 8 : g * 8 + 8],
                    sl.rearrange("p (k u) -> p k u", k=H))
            # ---- user-side weighted edge sums ----
            for ui in range(8):
                u = g * 8 + ui
                for h in range(NAH):
                    al_u = exp_g[:, h * 8 * H : (h + 1) * 8 * H].rearrange(
                        "p (k u) -> p k u", k=H)[:, :, ui]
                    nc.tensor.matmul(
                        misc[0:ED, EW_OFF + ui * H : EW_OFF + (ui + 1) * H],
                        edge_tiles[(ui, h)][:], al_u,
                        start=(h == 0), stop=(h == NAH - 1))
            nc.vector.tensor_copy(
                ew_all[:, g * 8 : g * 8 + 8, :].rearrange("p a b -> p (a b)"),
                misc[0:ED, EW_OFF : EW_OFF + 8 * H])

    # ---------- pass 3: ant-side sums and outputs ----------
    with tc.tile_pool(name="p3_sb", bufs=3) as p3_sb, \
         tc.tile_pool(name="p3_ps", bufs=2, space="PSUM") as p3_ps, \
         tc.tile_pool(name="po_ps", bufs=2, space="PSUM") as po_ps:
        # alpha_v2 (user-major) via direct [128,128] transposes of alpha_v3
        for j in range(NUC):
            for k in range(H):
                for h in range(NAH):
                    pt2 = p3_ps.tile([UC, 512], f32, tag="pt2")
                    nc.tensor.transpose(
                        pt2[:, 0:ACH],
                        alpha_v3[:, h, k, j * UC : (j + 1) * UC],
                        ident_sb[0:ACH, 0:ACH])
                    nc.scalar.copy(
                        alpha_v2[:, j, k, h * ACH : (h + 1) * ACH],
                        pt2[0:UC, 0:ACH])
        # ant-side weighted edge sums (contract over users); edge streamed
        # u-major in 8-antenna slabs
        edge_u4 = edge.rearrange("(j p a) e -> j p a e", p=UC, a=NA)
        for ag in range(NA // 8):
            ev = p3_sb.tile([UC, NUC, 8, ED], f32, tag="ev")
            for j in range(NUC):
                for ap2 in range(0, 8, 2):
                    nc.sync.dma_start(
                        ev[:, j, ap2 : ap2 + 2, :],
                        edge_u4[j, :, ag * 8 + ap2 : ag * 8 + ap2 + 2, :])
            pe = p3_ps.tile([ED, 512], f32, tag="pewa")
            for ai in range(8):
                a = ag * 8 + ai
                for j in range(NUC):
                    nc.tensor.matmul(
                        pe[:, ai * H : (ai + 1) * H],
                        ev[:, j, ai, :], alpha_v2[:, j, :, a],
                        start=(j == 0), stop=(j == NUC - 1))
            nc.vector.tensor_copy(
                ewa_all[:, ag * 8 : ag * 8 + 8, :].rearrange("p a b -> p (a b)"),
                pe[:, 0 : 8 * H])
        # user_out = concat_k(alpha@A_k + ew@We_k) + user@Wres
        uo_v = user_out.rearrange("(j p) d -> j p d", p=UC)
        for j in range(NUC):
            po = po_ps.tile([UC, HIDDEN], f32, tag="puo")
            for k in range(H):
                nc.tensor.matmul(po[:, k * HD : (k + 1) * HD],
                                 userT[:, j * UC : j * UC + UC],
                                 wres_sb[:, k * HD : (k + 1) * HD],
                                 start=True, stop=False)
                for h in range(NAH):
                    nc.tensor.matmul(
                        po[:, k * HD : (k + 1) * HD],
                        alpha_v3[:, h, k, j * UC : j * UC + UC],
                        A_big[:, h, EXTC + k * HD : EXTC + (k + 1) * HD],
                        start=False, stop=False)
                nc.tensor.matmul(
                    po[:, k * HD : (k + 1) * HD],
                    ew_all[:, j * UC : j * UC + UC, k],
                    we_big_sb[:, EXTC + k * HD : EXTC + (k + 1) * HD],
                    start=False, stop=True)
            ob = p3_sb.tile([UC, HIDDEN], f32, tag="ob")
            nc.scalar.copy(ob[:], po[:])
            nc.sync.dma_start(uo_v[j], ob[:])
        # ant_out = concat_k(alpha^T@U_k + ewa@We_k)
        ao_v = ant_out.rearrange("(i p) d -> i p d", p=ACH)
        for i in range(NA // ACH):
            po = po_ps.tile([ACH, HIDDEN], f32, tag="pao")
            for k in range(H):
                for j in range(NUC):
                    nc.tensor.matmul(
                        po[:, k * HD : (k + 1) * HD],
                        alpha_v2[:, j, k, i * ACH : (i + 1) * ACH],
                        U_big[:, j, EXTC + k * HD : EXTC + (k + 1) * HD],
                        start=(j == 0), stop=False)
                nc.tensor.matmul(
                    po[:, k * HD : (k + 1) * HD],
                    ewa_all[:, i * ACH : (i + 1) * ACH, k],
                    we_big_sb[:, EXTC + k * HD : EXTC + (k + 1) * HD],
                    start=False, stop=True)
            ob = p3_sb.tile([ACH, HIDDEN], f32, tag="ob2")
            nc.scalar.copy(ob[:], po[:])
            nc.sync.dma_start(ao_v[i], ob[:])


# ---------------------------------------------------------------------------
_CACHE = {}


def _get_nc(cfg):
    key = "nc"
    if key in _CACHE:
        return _CACHE[key]
    import concourse.bacc as bacc
    import concourse.mybir as mybir
    import concourse.tile as tile

    f32 = mybir.dt.float32
    nc = bacc.Bacc("TRN2", target_bir_lowering=False, debug=False)
    NU, NA, ED, UD, AD = cfg["NU"], cfg["NA"], cfg["ED"], cfg["UD"], cfg["AD"]
    EXTC, HH = cfg["EXTC"], cfg["HH"]
    ins = {
        "edge": nc.dram_tensor("edge", [NU * NA, ED], f32, kind="ExternalInput").ap(),
        "user": nc.dram_tensor("user", [NU, UD], f32, kind="ExternalInput").ap(),
        "ant": nc.dram_tensor("ant", [NA, AD], f32, kind="ExternalInput").ap(),
        "wu_big": nc.dram_tensor("wu_big", [UD, EXTC + HH], f32, kind="ExternalInput").ap(),
        "wa_big": nc.dram_tensor("wa_big", [AD, EXTC + HH], f32, kind="ExternalInput").ap(),
        "we_big": nc.dram_tensor("we_big", [ED, EXTC + HH], f32, kind="ExternalInput").ap(),
        "wres": nc.dram_tensor("wres", [UD, HH], f32, kind="ExternalInput").ap(),
        "ident": nc.dram_tensor("ident", [128, 128], f32, kind="ExternalInput").ap(),
    }
    outs = {
        "user_out": nc.dram_tensor("user_out", [NU, HH], f32, kind="ExternalOutput").ap(),
        "ant_out": nc.dram_tensor("ant_out", [NA, HH], f32, kind="ExternalOutput").ap(),
    }
    with tile.TileContext(nc) as tc:
        with ExitStack() as ctx:
            build_bgat(ctx, tc, outs, ins, cfg)
    nc.finalize()
    _CACHE[key] = nc
    return nc


_LAST_RES = {}


def kernel(user_feats, ant_feats, edge_feats, Wu, Wa, We, av, Wres,
           _trace=False):
    from concourse.bass_utils import run_bass_kernel_spmd

    user_feats = np.asarray(user_feats, np.float32)
    ant_feats = np.asarray(ant_feats, np.float32)
    edge_feats = np.asarray(edge_feats, np.float32)
    cfg = make_cfg(**FULL_CFG, av=av)
    wd = prep_weights(Wu, Wa, We, av, Wres, cfg)
    nc = _get_nc(cfg)
    NU, NA, ED = cfg["NU"], cfg["NA"], cfg["ED"]
    in_maps = []
    for b in range(B):
        in_maps.append({
            "edge": np.ascontiguousarray(edge_feats[b].reshape(NU * NA, ED)),
            "user": np.ascontiguousarray(user_feats[b]),
            "ant": np.ascontiguousarray(ant_feats[b]),
            "wu_big": wd["wu_big"], "wa_big": wd["wa_big"],
            "we_big": wd["we_big"], "wres": wd["wres"], "ident": wd["ident"],
        })
    res = run_bass_kernel_spmd(nc, in_maps, core_ids=list(range(B)),
                               trace=_trace)
    _LAST_RES["res"] = res
    user_out = np.stack([res.results[b]["user_out"] for b in range(B)])
    ant_out = np.stack([res.results[b]["ant_out"] for b in range(B)])
    return (user_out, ant_out)



# revision 2
# speedup vs baseline: 1.0908x; 1.0908x over previous
"""BGAT attention kernel v2.2 for Trainium2 (8 NeuronCores, batch-parallel).

vs v1 baseline:
  - score matmuls in bf16: PE cost is cycles_per_row=1 vs 4 for fp32, and
    cost scales only with output columns -- K rows are free
  - pos/neg column blocks are tight (no uniform-padding waste); T1 linear
    columns ride the pos bank, drained by the Act engine
  - A-term enters PSUM via two more bf16 matmuls (antT x wa-folds): 133ns
    on PE vs ~450ns as a DVE tensor_add (PSUM access latency dominates DVE)
  - one edge DMA per user ([128a, 2ah, 64e]); DMA queues split sync/gpsimd
"""

import math
from contextlib import ExitStack

import numpy as np

B = 8
NU, NA, ED, UD, AD = 256, 256, 64, 128, 128
H, HD = 8, 64
HH = H * HD
SCALE = 1.0 / math.sqrt(HD)


TRIM = 0


def make_cfg(av):
    av = np.asarray(av, np.float32)
    pos_idx = [np.nonzero(av[k] >= 0)[0] for k in range(H)]
    neg_idx = [np.nonzero(av[k] < 0)[0] for k in range(H)]
    if TRIM:
        def keep(ix, k):
            vals = np.abs(av[k][ix])
            order = np.argsort(vals)[TRIM:]  # drop TRIM smallest
            return ix[np.sort(order)]
        pos_idx = [keep(ix, k) for k, ix in enumerate(pos_idx)]
        neg_idx = [keep(ix, k) for k, ix in enumerate(neg_idx)]
    P_ = max(len(ix) for ix in pos_idx)
    N_ = max(len(ix) for ix in neg_idx)
    EXTP = H * P_ + H  # + t1 cols
    EXTN = H * N_
    assert EXTP <= 512 and EXTN <= 512
    return dict(P_=P_, N_=N_, EXTP=EXTP, EXTN=EXTN,
                pos_idx=pos_idx, neg_idx=neg_idx)


def prep_weights(Wu, Wa, We, av, Wres, cfg):
    import ml_dtypes
    Wu, Wa, We = (np.asarray(x, np.float32) for x in (Wu, Wa, We))
    av = np.asarray(av, np.float32)
    Wres = np.asarray(Wres, np.float32)
    P_, N_, EXTP, EXTN = cfg["P_"], cfg["N_"], cfg["EXTP"], cfg["EXTN"]

    weP = np.zeros((ED, EXTP), np.float32)
    weN = np.zeros((ED, EXTN), np.float32)
    wuP = np.zeros((UD, EXTP), np.float32)
    wuN = np.zeros((UD, EXTN), np.float32)
    waP = np.zeros((AD, EXTP), np.float32)
    waN = np.zeros((AD, EXTN), np.float32)
    for k in range(H):
        for i, d in enumerate(cfg["pos_idx"][k]):
            c = 0.4 * SCALE * abs(av[k, d])
            col = k * P_ + i
            weP[:, col] = We[k][:, d] * c
            wuP[:, col] = Wu[k][:, d] * c
            waP[:, col] = Wa[k][:, d] * c
        for i, d in enumerate(cfg["neg_idx"][k]):
            c = 0.4 * SCALE * abs(av[k, d])
            col = k * N_ + i
            weN[:, col] = We[k][:, d] * c
            wuN[:, col] = Wu[k][:, d] * c
            waN[:, col] = Wa[k][:, d] * c
        t1w = 0.6 * SCALE * av[k]
        col = H * P_ + k
        weP[:, col] = We[k] @ t1w
        wuP[:, col] = Wu[k] @ t1w
        waP[:, col] = Wa[k] @ t1w

    wu_raw = np.concatenate([Wu[k] for k in range(H)], axis=1)  # [UD, HH]
    wa_raw = np.concatenate([Wa[k] for k in range(H)], axis=1)
    we_raw = np.concatenate([We[k] for k in range(H)], axis=1)  # [ED, HH]

    bf = ml_dtypes.bfloat16
    return dict(
        weP_bf=weP.astype(bf), weN_bf=weN.astype(bf),
        waP_bf=waP.astype(bf), waN_bf=waN.astype(bf),
        wuP=wuP, wuN=wuN,
        wu_raw=wu_raw, wa_raw=wa_raw, we_raw=we_raw, wres=Wres,
        ident=np.eye(128, dtype=np.float32),
    )


def build_bgat2(ctx: ExitStack, tc, outs, ins, cfg):
    import concourse.mybir as mybir

    nc = tc.nc
    f32 = mybir.dt.float32
    bf16 = mybir.dt.bfloat16
    AX = mybir.AxisListType.X
    ADD = mybir.AluOpType.add
    EXPF = mybir.ActivationFunctionType.Exp

    P_, N_, EXTP, EXTN = cfg["P_"], cfg["N_"], cfg["EXTP"], cfg["EXTN"]
    NG = NU // 8

    edge = ins["edge"]
    user = ins["user"]
    ant = ins["ant"]
    user_out = outs["user_out"]
    ant_out = outs["ant_out"]

    consts = ctx.enter_context(tc.tile_pool(name="consts", bufs=1))
    ev_pool = ctx.enter_context(tc.tile_pool(name="ev_pool", bufs=12))

    def load_const(name, shape, dt=f32):
        t = consts.tile(list(shape), dt, name=f"c_{name}", tag=f"c_{name}")
        nc.sync.dma_start(t[:], ins[name][:, :])
        return t

    weP_sb = load_const("weP_bf", (ED, EXTP), bf16)
    weN_sb = load_const("weN_bf", (ED, EXTN), bf16)
    waP_sb = load_const("waP_bf", (AD, EXTP), bf16)
    waN_sb = load_const("waN_bf", (AD, EXTN), bf16)
    wuP_sb = load_const("wuP", (UD, EXTP))
    wuN_sb = load_const("wuN", (UD, EXTN))
    wu_raw_sb = load_const("wu_raw", (UD, HH))
    wa_raw_sb = load_const("wa_raw", (AD, HH))
    we_raw_sb = load_const("we_raw", (ED, HH))
    wres_sb = load_const("wres", (UD, HH))
    ident_sb = load_const("ident", (128, 128))

    ones_col = consts.tile([128, 1], f32)
    nc.gpsimd.memset(ones_col[:], 1.0)
    ones_row = consts.tile([1, 128], f32)
    nc.gpsimd.memset(ones_row[:], 1.0)

    userT = consts.tile([UD, NU], f32)
    antT_bf = consts.tile([AD, NA], bf16)
    U_extPN = consts.tile([128, 2, EXTP + EXTN], bf16)
    U_raw = consts.tile([128, 2, HH], f32)
    A_raw = consts.tile([128, 2, HH], f32)
    alpha_v3 = consts.tile([128, 2, NU, H], f32)   # [a, ah, u, k]
    alpha_v2 = consts.tile([128, 2, H, NA], f32)   # [u, j, k, a]
    ew_all = consts.tile([ED, NU, H], f32)
    ewa_all = consts.tile([ED, NA, H], f32)

    # ---- precompute ----
    with tc.tile_pool(name="pre_sb", bufs=2) as pre_sb, \
         tc.tile_pool(name="pre_ps", bufs=2, space="PSUM") as pre_ps:
        antT32 = pre_sb.tile([AD, NA], f32, tag="antT32", bufs=1)
        for (feat, T_sb, fd) in ((user, userT, UD), (ant, antT32, AD)):
            ft = pre_sb.tile([128, 2, fd], f32, tag="ft")
            nc.sync.dma_start(ft[:], feat.rearrange("(j p) f -> p j f", p=128))
            for j in range(2):
                pt = pre_ps.tile([fd, 128], f32, tag="pt")
                nc.tensor.transpose(pt[:], ft[:, j, :], ident_sb[:])
                nc.scalar.copy(T_sb[:, j * 128:(j + 1) * 128], pt[:])
        nc.vector.tensor_copy(antT_bf[:], antT32[:])
        for (T_sb, dst, w_sb, w, c0) in (
            (userT, U_extPN, wuP_sb, EXTP, 0), (userT, U_extPN, wuN_sb, EXTN, EXTP),
            (userT, U_raw, wu_raw_sb, HH, 0),
            (antT32, A_raw, wa_raw_sb, HH, 0),
        ):
            for j in range(2):
                ps = pre_ps.tile([128, 512], f32, tag="proj")
                nc.tensor.matmul(ps[:, 0:w], T_sb[:, j * 128:(j + 1) * 128],
                                 w_sb[:, 0:w], start=True, stop=True)
                nc.scalar.copy(dst[:, j, c0:c0 + w], ps[:, 0:w])

    # ---- pass 1: scores + softmax + user-side ew ----
    SUM_OFF, RB_OFF, EW_OFF = 0, 64, 128  # offsets inside misc bank
    with tc.tile_pool(name="edge_pool", bufs=4) as edge_pool, \
         tc.tile_pool(name="lhs_pool", bufs=4) as lhs_pool, \
         tc.tile_pool(name="combo_pool", bufs=1) as combo_pool, \
         tc.tile_pool(name="stage_pool", bufs=3) as stage_pool, \
         tc.tile_pool(name="ps_tp", bufs=1, space="PSUM") as ps_tp, \
         tc.tile_pool(name="ps_P", bufs=3, space="PSUM") as ps_P, \
         tc.tile_pool(name="ps_N", bufs=2, space="PSUM") as ps_N, \
         tc.tile_pool(name="ps_misc", bufs=1, space="PSUM") as ps_misc, \
         tc.tile_pool(name="ps_ew", bufs=1, space="PSUM") as ps_ew:

        edge_v = edge.rearrange("(u ah p) e -> u p ah e", ah=2, p=128)
        edge_u4 = edge.rearrange("(j p a) e -> j p a e", p=128, a=NA)
        ev_tiles = {}

        combo_slots = []
        for s in range(8):
            cb = combo_pool.tile([ED + 1, EXTP + EXTN], bf16, tag=f"cb{s}")
            nc.sync.dma_start(cb[0:ED, 0:EXTP], weP_sb[:])
            nc.sync.dma_start(cb[0:ED, EXTP:], weN_sb[:])
            combo_slots.append(cb)

        for g in range(NG):
            misc = ps_misc.tile([128, 512], f32, tag="misc")
            stage_PN = stage_pool.tile([128, 16, 2, H], f32, tag="sPN")
            stage_T = stage_pool.tile([128, 16, H], f32, tag="sT")
            edge_tiles = {}
            for ui in range(8):
                u = g * 8 + ui
                et = edge_pool.tile([128, 2, ED], f32, tag=f"e{ui}")
                (nc.sync, nc.scalar)[ui % 2].dma_start(et[:], edge_v[u])
                edge_tiles[ui] = et
                cb = combo_slots[ui]
                nc.sync.dma_start(cb[ED:ED + 1, :],
                                  U_extPN[u % 128:u % 128 + 1, u // 128, :])
            for bi in range(4):
                ah = bi // 2
                us = (bi % 2) * 4
                tp = ps_tp.tile([ED, 512], f32, tag="tp")
                lhs = lhs_pool.tile([ED + 1, 512], bf16, tag="lhs")
                nc.gpsimd.memset(lhs[ED:ED + 1, :], 1.0)
                for q in range(4):
                    nc.tensor.transpose(tp[:, q * 128:(q + 1) * 128],
                                        edge_tiles[us + q][:, ah, :],
                                        ident_sb[:])
                nc.scalar.copy(lhs[0:ED, :], tp[:])
                aslab = antT_bf[:, ah * 128:(ah + 1) * 128]
                for q in range(4):
                    ui = us + q
                    cb = combo_slots[ui]
                    sidx = ah * 8 + ui
                    lh = lhs[:, q * 128:(q + 1) * 128]
                    psP = ps_P.tile([128, EXTP], f32, tag="P")
                    psN = ps_N.tile([128, EXTN], f32, tag="N")
                    nc.tensor.matmul(psP[:], lh, cb[:, 0:EXTP],
                                     start=True, stop=False)
                    nc.tensor.matmul(psP[:], aslab, waP_sb[:],
                                     start=False, stop=True)
                    nc.tensor.matmul(psN[:], lh, cb[:, EXTP:],
                                     start=True, stop=False)
                    nc.tensor.matmul(psN[:], aslab, waN_sb[:],
                                     start=False, stop=True)
                    nc.vector.tensor_reduce(
                        stage_PN[:, sidx, 0, :],
                        psP[:, 0:H * P_].rearrange("p (k d) -> p k d", d=P_),
                        axis=AX, op=ADD, apply_absolute_value=True)
                    nc.vector.tensor_reduce(
                        stage_PN[:, sidx, 1, :],
                        psN[:, 0:H * N_].rearrange("p (k d) -> p k d", d=N_),
                        axis=AX, op=ADD, apply_absolute_value=True)
                    nc.scalar.copy(stage_T[:, sidx, :],
                                   psP[:, H * P_:H * P_ + H])

            # softmax over antennas
            score = stage_pool.tile([128, 16, H], f32, tag="score")
            nc.gpsimd.tensor_sub(score[:], stage_PN[:, :, 0, :],
                                 stage_PN[:, :, 1, :])
            nc.gpsimd.tensor_add(score[:], score[:], stage_T[:])
            expg = stage_pool.tile([128, 16, H], f32, tag="expg")
            nc.scalar.activation(expg[:], score[:], EXPF)
            for ah in range(2):
                nc.tensor.matmul(
                    misc[0:1, SUM_OFF:SUM_OFF + 64], ones_col[:],
                    expg[:, ah * 8:(ah + 1) * 8, :].rearrange("p a b -> p (a b)"),
                    start=(ah == 0), stop=(ah == 1))
            rec = stage_pool.tile([1, 64], f32, tag="rec")
            nc.vector.reciprocal(rec[:], misc[0:1, SUM_OFF:SUM_OFF + 64])
            nc.tensor.matmul(misc[:, RB_OFF:RB_OFF + 64], ones_row[:], rec[:],
                             start=True, stop=True)
            for ah in range(2):
                nc.vector.tensor_mul(
                    alpha_v3[:, ah, g * 8:g * 8 + 8, :],
                    expg[:, ah * 8:(ah + 1) * 8, :],
                    misc[:, RB_OFF:RB_OFF + 64].rearrange(
                        "p (a b) -> p a b", a=8))

            # user-side weighted edge sums
            ewb = ps_ew.tile([ED, 8 * H], f32, tag="ewb")
            for ui in range(8):
                u = g * 8 + ui
                for ah in range(2):
                    nc.tensor.matmul(
                        ewb[:, ui * H:(ui + 1) * H],
                        edge_tiles[ui][:, ah, :],
                        alpha_v3[:, ah, u, :],
                        start=(ah == 0), stop=(ah == 1))
            nc.scalar.copy(
                ew_all[:, g * 8:g * 8 + 8, :].rearrange("p a b -> p (a b)"),
                ewb[:])
            # prefetch pass-3 slab (no deps; overlaps with pass-1)
            ev = ev_pool.tile([128, 2, 8, ED], f32, tag="ev", name=f"ev{g}")
            for j in range(2):
                nc.scalar.dma_start(
                    ev[:, j, :, :],
                    edge_u4[j, :, g * 8:g * 8 + 8, :])
            ev_tiles[g] = ev

    # ---- pass 3: ant-side sums + outputs ----
    with tc.tile_pool(name="p3_sb", bufs=3) as p3_sb, \
         tc.tile_pool(name="p3_ps", bufs=2, space="PSUM") as p3_ps, \
         tc.tile_pool(name="po_ps", bufs=2, space="PSUM") as po_ps:
        # alpha_v2 via transposes, 4 per bank
        quads = [(j, ah, k) for j in range(2) for ah in range(2)
                 for k in range(H)]
        for q0 in range(0, len(quads), 4):
            pt = p3_ps.tile([128, 512], f32, tag="pt2")
            for qq, (j, ah, k) in enumerate(quads[q0:q0 + 4]):
                nc.tensor.transpose(
                    pt[:, qq * 128:(qq + 1) * 128],
                    alpha_v3[:, ah, j * 128:(j + 1) * 128, k], ident_sb[:])
            for qq, (j, ah, k) in enumerate(quads[q0:q0 + 4]):
                nc.scalar.copy(alpha_v2[:, j, k, ah * 128:(ah + 1) * 128],
                               pt[0:128, qq * 128:(qq + 1) * 128])
        # ant-side: slabs were prefetched during pass 1
        for ag in range(NA // 8):
            ev = ev_tiles[ag]
            pe = p3_ps.tile([ED, 512], f32, tag="pewa")
            for ai in range(8):
                a = ag * 8 + ai
                for j in range(2):
                    nc.tensor.matmul(
                        pe[:, ai * H:(ai + 1) * H],
                        ev[:, j, ai, :], alpha_v2[:, j, :, a],
                        start=(j == 0), stop=(j == 1))
            nc.scalar.copy(
                ewa_all[:, ag * 8:ag * 8 + 8, :].rearrange("p a b -> p (a b)"),
                pe[:, 0:8 * H])
        # user_out
        uo_v = user_out.rearrange("(j p) d -> j p d", p=128)
        for j in range(2):
            po = po_ps.tile([128, HH], f32, tag="puo")
            for k in range(H):
                cs = slice(k * HD, (k + 1) * HD)
                nc.tensor.matmul(po[:, cs], userT[:, j * 128:(j + 1) * 128],
                                 wres_sb[:, cs], start=True, stop=False)
                for ah in range(2):
                    nc.tensor.matmul(
                        po[:, cs],
                        alpha_v3[:, ah, j * 128:(j + 1) * 128, k],
                        A_raw[:, ah, cs], start=False, stop=False)
                nc.tensor.matmul(po[:, cs],
                                 ew_all[:, j * 128:(j + 1) * 128, k],
                                 we_raw_sb[:, cs], start=False, stop=True)
            ob = p3_sb.tile([128, HH], f32, tag="ob")
            nc.scalar.copy(ob[:], po[:])
            nc.sync.dma_start(uo_v[j], ob[:])
        # ant_out
        ao_v = ant_out.rearrange("(i p) d -> i p d", p=128)
        for i in range(2):
            po = po_ps.tile([128, HH], f32, tag="pao")
            for k in range(H):
                cs = slice(k * HD, (k + 1) * HD)
                for j in range(2):
                    nc.tensor.matmul(
                        po[:, cs], alpha_v2[:, j, k, i * 128:(i + 1) * 128],
                        U_raw[:, j, cs], start=(j == 0), stop=False)
                nc.tensor.matmul(po[:, cs],
                                 ewa_all[:, i * 128:(i + 1) * 128, k],
                                 we_raw_sb[:, cs], start=False, stop=True)
            ob = p3_sb.tile([128, HH], f32, tag="ob2")
            nc.scalar.copy(ob[:], po[:])
            nc.sync.dma_start(ao_v[i], ob[:])


# ---------------------------------------------------------------------------
_CACHE = {}

IN_NAMES = ("weP_bf", "weN_bf", "waP_bf", "waN_bf", "wuP", "wuN",
            "wu_raw", "wa_raw", "we_raw", "wres", "ident")


def get_nc(cfg, reps=1):
    import concourse.bacc as bacc
    import concourse.mybir as mybir
    import concourse.tile as tile

    f32 = mybir.dt.float32
    bf16 = mybir.dt.bfloat16
    EXTP, EXTN = cfg["EXTP"], cfg["EXTN"]
    nc = bacc.Bacc("TRN2", target_bir_lowering=False, debug=False)
    shapes = dict(weP_bf=([ED, EXTP], bf16), weN_bf=([ED, EXTN], bf16),
                  waP_bf=([AD, EXTP], bf16), waN_bf=([AD, EXTN], bf16),
                  wuP=([UD, EXTP], f32), wuN=([UD, EXTN], f32),
                  wu_raw=([UD, HH], f32), wa_raw=([AD, HH], f32),
                  we_raw=([ED, HH], f32), wres=([UD, HH], f32),
                  ident=([128, 128], f32))
    ins = {
        "edge": nc.dram_tensor("edge", [NU * NA, ED], f32, kind="ExternalInput").ap(),
        "user": nc.dram_tensor("user", [NU, UD], f32, kind="ExternalInput").ap(),
        "ant": nc.dram_tensor("ant", [NA, AD], f32, kind="ExternalInput").ap(),
    }
    for n, (shp, dt) in shapes.items():
        ins[n] = nc.dram_tensor(n, shp, dt, kind="ExternalInput").ap()
    outs = {
        "user_out": nc.dram_tensor("user_out", [NU, HH], f32, kind="ExternalOutput").ap(),
        "ant_out": nc.dram_tensor("ant_out", [NA, HH], f32, kind="ExternalOutput").ap(),
    }
    with tile.TileContext(nc) as tc:
        for _ in range(reps):
            with ExitStack() as ctx:
                build_bgat2(ctx, tc, outs, ins, cfg)
    nc.finalize()
    return nc


def make_in_map(inputs, b, cfg, wd):
    return {
        "edge": np.ascontiguousarray(
            np.asarray(inputs["edge_feats"][b], np.float32).reshape(NU * NA, ED)),
        "user": np.ascontiguousarray(np.asarray(inputs["user_feats"][b], np.float32)),
        "ant": np.ascontiguousarray(np.asarray(inputs["ant_feats"][b], np.float32)),
        **{k: wd[k] for k in IN_NAMES},
    }


_LAST_RES = {}


def kernel(user_feats, ant_feats, edge_feats, Wu, Wa, We, av, Wres,
           _trace=False):
    from concourse.bass_utils import run_bass_kernel_spmd

    inputs = dict(user_feats=user_feats, ant_feats=ant_feats,
                  edge_feats=edge_feats)
    cfg = make_cfg(av)
    wd = prep_weights(Wu, Wa, We, av, Wres, cfg)
    key = ("nc", cfg["P_"], cfg["N_"])
    if key not in _CACHE:
        _CACHE[key] = get_nc(cfg)
    nc = _CACHE[key]
    in_maps = [make_in_map(inputs, b, cfg, wd) for b in range(B)]
    res = run_bass_kernel_spmd(nc, in_maps, core_ids=list(range(B)),
                               trace=_trace)
    _LAST_RES["res"] = res
    user_out = np.stack([res.results[b]["user_out"] for b in range(B)])
    ant_out = np.stack([res.results[b]["ant_out"] for b in range(B)])
    return (user_out, ant_out)
